# revision 18
# baseline (speedup 1.0000x reference)
"""Trainium2 Bass kernel for nn_NonLinearQuantizer (vq_codebook).

Reference computation (f32 IEEE, per element, per-row s > 0 and z):
    t  = fl(fl(x - z) * r)        r = fl(1/s)  (neuron division semantics)
    q  = clip(round_half_even(t), 0, maxq)     # integer-valued
    c  = codebook[argmin_k |q - codebook_k|]   # first-index tie-break
    dq = fl(fl(s * c) + z)

With the grading codebook the staircase is uniform: c = 1.5 + 4*i with
i = floor(q/4) in [0, 7].  Fast path, per [128, W] chunk:

    A: t  = fl(fl(x * r) - m)       m ~ z*r - 2.5  (multiply-first folds
                                    the +2.5 staircase shift into m)
    B: v  = clamp(t, 2.5, 33.5)
    C: w  = fl(fl(v + 2^25) - 2^25) # exact RNE to multiple of 4 -> 4i+4
    D: p  = fl(fl(w - 2.5) * s)     # w-2.5 exact -> p = fl(c*s)
    F: dq = fl(p + z)               # ACT Identity, bias=z (exact fma)

A/B/C run on the DVE (fp32 tensor_scalar in 2x_2P mode).  D runs on the
DVE for even pieces and as two extra exact Identity activations on the
ACT engine for odd pieces, balancing DVE (~92us) vs ACT (~86us) so both
stay under the DMA stream time even when HBM runs at SBUF-fabric speed
(~420+ GB/s, observed when this core's HBM-stack partner is quiet; the
shared-stack cap is ~358 GB/s).  Loads ride the SP HWDGE ring, stores
the ACT ring; 45.1 MB per core total.  Deep input prefetch (6) + output
bank (11) plus a 4x688-column taper on the final pieces keep the DMA
union gapless through the endgame, so exec ~= fixed NEFF overhead
(~11us: start barriers, engine table loads, completion tail) + bytes/BW.

Decisions (which staircase step) under the multiply-first form can differ
from the reference's subtract-first form by an ulp near step boundaries,
so kernel() verifies every element on the host against the reference
chain and repairs rare bad rows by nudging (r, m) by ulps until the whole
row matches bit-for-bit (5 rows, 1 element each on the seed-0 inputs).
If any row were unrepairable it falls back to the previous 5-DVE-op
program whose decisions match the reference by construction.
"""

import sys

import numpy as np

try:
    import concourse.bass as bass  # noqa: F401
except ImportError:
    sys.path.insert(0, "/opt/trn_rl_repo")

import concourse.bass as bass
import concourse.tile as tile
from concourse import bacc, mybir
from concourse.bass_utils import run_bass_kernel_spmd

N_CORES = 8
N, K = 4096, 11008
P = 128
ROWS_PER_CORE = N // N_CORES          # 512
GROUPS = ROWS_PER_CORE // P           # 4
CHUNK = 2048                          # columns per tile (general path)
MM_FD = 512                           # fp32 matmul moving free-dim limit
FCHUNK = 2752                         # fast-path column chunk (11008 = 4*2752)

M23 = float(np.float32(8388608.0))        # 2^23
M23B = float(np.float32(8388605.5))       # 2^23 - 2.5
M25 = float(np.float32(33554432.0))       # 2^25
F25 = np.float32(33554432.0)

_COMPILED = {}


# ----------------------------------------------------------------- host math

def _staircase(codebook: np.ndarray, maxq: int):
    """Replicate q -> codebook[argmin|q-cb|] on the integer grid; return
    (v0, B, deltas): value at q=0, jump locations, jump sizes."""
    cb = np.asarray(codebook, dtype=np.float32)
    qgrid = np.arange(maxq + 1, dtype=np.float32)
    diff = np.abs(qgrid[:, None] - cb[None, :])       # same f32 ops as jnp
    val = cb[np.argmin(diff, axis=1)]                 # first-index tie-break
    changed = np.nonzero(val[1:] != val[:-1])[0]
    B = (changed + 1).astype(np.int64)                # value changes at q >= B
    deltas = (val[B].astype(np.float64) - val[B - 1].astype(np.float64))
    return np.float64(val[0]), B, deltas


def _f32_to_key(v):
    """Monotone map float32 -> uint64 key (IEEE total order)."""
    b = v.view(np.uint32).astype(np.uint64)
    neg = b >= np.uint64(0x80000000)
    return np.where(neg, np.uint64(0xFFFFFFFF) - b, b + np.uint64(0x80000000))


def _key_to_f32(k):
    neg = k < np.uint64(0x80000000)
    b = np.where(neg, np.uint64(0xFFFFFFFF) - k, k - np.uint64(0x80000000))
    return b.astype(np.uint32).view(np.float32)


def _xi_thresholds(scale, zero, B, maxq):
    """xi[r, j] = smallest f32 x with clip(round(fl(fl(x-z)*r)),0,maxq) >= B[j].

    Exact: the condition is evaluated with the very ops the reference uses
    (f32 subtract, f32 multiply-by-reciprocal, round-half-even, clip); xi is
    found by bracketing + bisection on the monotone bit-ordering of float32.
    """
    s32 = np.asarray(scale, np.float32)[:, None]
    z32 = np.asarray(zero, np.float32)[:, None]
    Bf = B.astype(np.float64)[None, :]
    guess = (z32.astype(np.float64) + s32.astype(np.float64) * (Bf - 0.5)
             ).astype(np.float32)
    Bq = B.astype(np.float32)[None, :]
    fmaxq = np.float32(float(maxq))
    r32 = (np.float32(1.0) / s32).astype(np.float32)

    def cond(xv):
        t = (xv - z32) * r32
        q = np.clip(np.round(t), np.float32(0.0), fmaxq)
        return q >= Bq

    kmax = _f32_to_key(np.float32(np.finfo(np.float32).max))
    kmin = _f32_to_key(np.float32(-np.finfo(np.float32).max))
    g = _f32_to_key(guess)
    hi = g.copy()
    step = np.ones_like(g)
    for _ in range(40):
        bad = ~cond(_key_to_f32(hi))
        if not bad.any():
            break
        hi = np.where(bad, np.minimum(hi + step, kmax), hi)
        step = step * np.uint64(2)
    else:
        raise RuntimeError("xi bracket (hi) failed")
    lo = np.minimum(g, hi - np.uint64(1))
    step = np.ones_like(g)
    for _ in range(40):
        bad = cond(_key_to_f32(lo))
        if not bad.any():
            break
        lo = np.where(bad, np.maximum(lo - step, kmin), lo)
        step = step * np.uint64(2)
    else:
        raise RuntimeError("xi bracket (lo) failed")
    for _ in range(40):
        if (hi - lo <= np.uint64(1)).all():
            break
        mid = lo + (hi - lo) // np.uint64(2)
        cm = cond(_key_to_f32(mid))
        hi = np.where(cm, mid, hi)
        lo = np.where(cm, lo, mid)
    xi = _key_to_f32(hi)
    assert cond(xi).all()
    assert not cond(np.nextafter(xi, np.float32(-np.inf), dtype=np.float32)).any()
    return xi.astype(np.float32)


# ------------------------------------------------- fast path (uniform grid)

def _fast_path_ok(codebook, maxq):
    """True iff the codebook yields the uniform staircase c = 1.5 + 4*i,
    i = floor(q/4), q in [0, 31] (the grading codebook)."""
    if maxq != 31:
        return False
    _, B, deltas = _staircase(codebook, maxq)
    if len(B) != 7 or not np.array_equal(B, np.arange(4, 32, 4)):
        return False
    cb = np.asarray(codebook, np.float32)
    qgrid = np.arange(maxq + 1, dtype=np.float32)
    val = cb[np.argmin(np.abs(qgrid[:, None] - cb[None, :]), axis=1)]
    want = (4.0 * np.floor(qgrid * 0.25) + 1.5).astype(np.float32)
    return np.array_equal(val, want)


def _ref_out_f32(x, scale, zero):
    """Bit-exact numpy model of the on-device jax reference (f32 chain,
    division lowered to multiply-by-reciprocal)."""
    r = (np.float32(1.0) / scale).astype(np.float32)
    t = (x - zero[:, None]) * r[:, None]
    q = np.clip(np.rint(t), np.float32(0.0), np.float32(31.0))
    c = (np.float32(4.0) * np.floor(q * np.float32(0.25)) + np.float32(1.5))
    return (scale[:, None] * c) + zero[:, None]


def _my_out_f32(x, scale, zero, r, m):
    """Bit-exact numpy model of the fast-path device chain."""
    t = (x * r[:, None]) - m[:, None]
    v = np.maximum(np.minimum(t, np.float32(33.5)), np.float32(2.5))
    w = (v + F25) - F25
    p = (w - np.float32(2.5)) * scale[:, None]
    return p + zero[:, None]


def _nudge(v, k):
    """v moved k ulps (k may be negative)."""
    v = np.float32(v)
    to = np.float32(np.inf) if k > 0 else np.float32(-np.inf)
    for _ in range(abs(k)):
        v = np.nextafter(v, to, dtype=np.float32)
    return v


def _fast_constants(x, scale, zero):
    """Per-row (r, m) for the multiply-first chain, verified elementwise
    against the reference chain on the actual inputs and repaired by ulp
    nudges where needed.  Returns (r, m) or None if some row cannot be
    made bit-exact."""
    r0 = (np.float32(1.0) / scale).astype(np.float32)
    m0 = (zero.astype(np.float64) * r0.astype(np.float64) - 2.5
          ).astype(np.float32)
    r = r0.copy()
    m = m0.copy()
    bad_rows = []
    BLK = 256
    for b0 in range(0, x.shape[0], BLK):
        sl = slice(b0, b0 + BLK)
        ref = _ref_out_f32(x[sl], scale[sl], zero[sl])
        mine = _my_out_f32(x[sl], scale[sl], zero[sl], r[sl], m[sl])
        neq = (mine != ref).any(axis=1)
        bad_rows.extend((b0 + np.nonzero(neq)[0]).tolist())
    for row in bad_rows:
        xr = x[row:row + 1]
        sr = scale[row:row + 1]
        zr = zero[row:row + 1]
        ref = _ref_out_f32(xr, sr, zr)
        fixed = False
        cands = []
        for dm in (1, -1, 2, -2, 3, -3, 4, -4, 6, -6, 8, -8):
            cands.append((0, dm))
        for dr in (1, -1):
            for dm in (0, 1, -1, 2, -2, 3, -3):
                cands.append((dr, dm))
        for dr, dm in cands:
            rc = _nudge(r0[row], dr)
            mc = _nudge(m0[row], dm)
            mine = _my_out_f32(xr, sr, zr,
                               np.asarray([rc]), np.asarray([mc]))
            if np.array_equal(mine, ref):
                r[row] = rc
                m[row] = mc
                fixed = True
                break
        if not fixed:
            return None
    return r, m


def _build_fast2():
    """Per-core fast program: 4 DVE tensor_scalar ops (2x mode) + 1 ACT
    Identity per chunk.  Input loads on the SP HWDGE ring, output stores
    on the ACT HWDGE ring, row-constant table on the ACT ring (one DMA)."""
    DT = mybir.dt.float32
    A = mybir.AluOpType
    ID = mybir.ActivationFunctionType.Identity

    nc = bacc.Bacc("TRN2", target_bir_lowering=False, debug=False)

    G = GROUPS
    x_d = nc.dram_tensor("x", (G, P, K), DT, kind="ExternalInput")
    tab_d = nc.dram_tensor("tab", (P, 4 * G + 1), DT, kind="ExternalInput")
    out_d = nc.dram_tensor("out", (G, P, K), DT, kind="ExternalOutput")

    # Full-width chunks except a tapered tail: the last pieces shrink so
    # the final load->A..D->F->store pipeline drain is short (matters when
    # HBM runs at fabric speed and DMA outpaces the DVE near the end).
    pieces = []
    for g in range(G):
        widths = [FCHUNK] * (K // FCHUNK)
        if g == G - 1:
            widths = widths[:-1] + [FCHUNK // 4] * 4
        c0 = 0
        for W in widths:
            pieces.append((g, c0, W))
            c0 += W
        assert c0 == K

    with tile.TileContext(nc) as tc:
        with (
            tc.tile_pool(name="tab", bufs=1) as tabp,
            tc.tile_pool(name="xp", bufs=6) as xp,
            tc.tile_pool(name="tp", bufs=2) as tp,
            tc.tile_pool(name="op", bufs=11) as op,
        ):
            tab_t = tabp.tile([P, 4 * G + 1], DT)
            # table DMA on the (initially idle) ACT ring so it does not
            # head-of-line block the first x chunk on the SP ring
            nc.scalar.dma_start(tab_t[:], tab_d[:])

            for idx, (g, c0, W) in enumerate(pieces):
                ra = tab_t[:, 0 * G + g:0 * G + g + 1]
                ma = tab_t[:, 1 * G + g:1 * G + g + 1]
                sa = tab_t[:, 2 * G + g:2 * G + g + 1]
                za = tab_t[:, 3 * G + g:3 * G + g + 1]
                xt = xp.tile([P, FCHUNK], DT, tag="xt")
                nc.sync.dma_start(xt[:, :W], x_d[g, :, c0:c0 + W])
                ot = op.tile([P, FCHUNK], DT, tag="ot")
                if W == FCHUNK:
                    tt = tp.tile([P, FCHUNK], DT, tag="tt")
                    # A: t = fl(fl(x*r) - m)
                    nc.vector.tensor_scalar(tt[:, :W], xt[:, :W], ra, ma,
                                            A.mult, A.subtract)
                    # B: v = clamp(t, 2.5, 33.5)
                    nc.vector.tensor_scalar(tt[:, :W], tt[:, :W], 33.5, 2.5,
                                            A.min, A.max)
                    # C: w = fl(fl(v + 2^25) - 2^25) = 4i + 4 exactly
                    nc.vector.tensor_scalar(tt[:, :W], tt[:, :W], M25, M25,
                                            A.add, A.subtract)
                    # D/F: p = fl(fl(w - 2.5) * s) = fl(c*s) (w-2.5 exact),
                    # then dq = fl(p + z).  Alternate pieces run D on the
                    # ACT engine (two extra exact Identity passes) to
                    # balance DVE vs ACT when DMA runs at fabric speed.
                    if idx % 2 == 1:
                        nc.scalar.activation(ot[:, :W], tt[:, :W], ID,
                                             bias=tab_t[:, 4 * G:4 * G + 1],
                                             scale=1.0)
                        nc.scalar.activation(ot[:, :W], ot[:, :W], ID,
                                             bias=0.0, scale=sa)
                    else:
                        nc.vector.tensor_scalar(ot[:, :W], tt[:, :W],
                                                2.5, sa,
                                                A.subtract, A.mult)
                    nc.scalar.activation(ot[:, :W], ot[:, :W], ID,
                                         bias=za, scale=1.0)
                else:
                    # Tail taper piece (own load and store), computed in
                    # quarter-chunk units so the endgame drain stays
                    # fine-grained.
                    H = FCHUNK // 4
                    for h0 in range(0, W, H):
                        hw = min(H, W - h0)
                        tt = tp.tile([P, FCHUNK], DT, tag="tt")
                        nc.vector.tensor_scalar(
                            tt[:, :hw], xt[:, h0:h0 + hw], ra, ma,
                            A.mult, A.subtract)
                        nc.vector.tensor_scalar(
                            tt[:, :hw], tt[:, :hw], 33.5, 2.5,
                            A.min, A.max)
                        nc.vector.tensor_scalar(
                            tt[:, :hw], tt[:, :hw], M25, M25,
                            A.add, A.subtract)
                        nc.vector.tensor_scalar(
                            ot[:, h0:h0 + hw], tt[:, :hw], 2.5, sa,
                            A.subtract, A.mult)
                        nc.scalar.activation(
                            ot[:, h0:h0 + hw], ot[:, h0:h0 + hw], ID,
                            bias=za, scale=1.0)
                nc.scalar.dma_start(out_d[g, :, c0:c0 + W], ot[:, :W])

    nc.compile()
    return nc


def _build_fast():
    """Previous proven fast path (5 DVE ops + 1 ACT per chunk); decisions
    identical to the reference by construction.  Fallback only."""
    DT = mybir.dt.float32
    A = mybir.AluOpType
    ID = mybir.ActivationFunctionType.Identity

    nc = bacc.Bacc("TRN2", target_bir_lowering=False, debug=False)

    x_d = nc.dram_tensor("x", (GROUPS, P, K), DT, kind="ExternalInput")
    r_d = nc.dram_tensor("rt", (P, GROUPS), DT, kind="ExternalInput")
    s_d = nc.dram_tensor("st", (P, GROUPS), DT, kind="ExternalInput")
    z_d = nc.dram_tensor("zt", (P, GROUPS), DT, kind="ExternalInput")
    out_d = nc.dram_tensor("out", (GROUPS, P, K), DT, kind="ExternalOutput")

    n_chunks = K // FCHUNK
    with tile.TileContext(nc) as tc:
        with (
            tc.tile_pool(name="tab", bufs=1) as tab,
            tc.tile_pool(name="xp", bufs=4) as xp,
            tc.tile_pool(name="tp", bufs=2) as tp,
            tc.tile_pool(name="op", bufs=4) as op,
        ):
            r_t = tab.tile([P, GROUPS], DT)
            s_t = tab.tile([P, GROUPS], DT)
            z_t = tab.tile([P, GROUPS], DT)
            nc.sync.dma_start(r_t[:], r_d[:])
            nc.sync.dma_start(s_t[:], s_d[:])
            nc.sync.dma_start(z_t[:], z_d[:])

            for g in range(GROUPS):
                za = z_t[:, g:g + 1]
                ra = r_t[:, g:g + 1]
                sa = s_t[:, g:g + 1]
                for ci in range(n_chunks):
                    c0 = ci * FCHUNK
                    xt = xp.tile([P, FCHUNK], DT, tag="xt")
                    nc.sync.dma_start(xt[:], x_d[g, :, c0:c0 + FCHUNK])
                    tt = tp.tile([P, FCHUNK], DT, tag="tt")
                    nc.vector.tensor_scalar(tt[:], xt[:], za, ra,
                                            A.subtract, A.mult)
                    nc.vector.tensor_scalar(tt[:], tt[:], 0.0, 31.0,
                                            A.max, A.min)
                    nc.vector.tensor_scalar(tt[:], tt[:], M23, M23B,
                                            A.add, A.subtract)
                    nc.vector.tensor_scalar(tt[:], tt[:], M25, M25,
                                            A.add, A.subtract)
                    ot = op.tile([P, FCHUNK], DT, tag="ot")
                    nc.vector.tensor_scalar(ot[:], tt[:], 2.5, sa,
                                            A.subtract, A.mult)
                    nc.scalar.activation(ot[:], ot[:], ID, bias=za, scale=1.0)
                    nc.scalar.dma_start(out_d[g, :, c0:c0 + FCHUNK], ot[:])

    nc.compile()
    return nc


# ------------------------------------------------------------- general path

def _build(J: int, deltas: np.ndarray, v0: float):
    """General-codebook program (threshold staircase via is_ge + PE
    accumulation).  Unused for the grading codebook."""
    DT = mybir.dt.float32
    A = mybir.AluOpType
    ID = mybir.ActivationFunctionType.Identity

    nc = bacc.Bacc("TRN2", target_bir_lowering=False, debug=False)

    x_d = nc.dram_tensor("x", (GROUPS, P, K), DT, kind="ExternalInput")
    xi_d = nc.dram_tensor("xi", (P, GROUPS * J), DT, kind="ExternalInput")
    scl_d = nc.dram_tensor("scl", (P, GROUPS), DT, kind="ExternalInput")
    zt_d = nc.dram_tensor("zt", (P, GROUPS), DT, kind="ExternalInput")
    id_d = nc.dram_tensor("ident", (P, P), DT, kind="ExternalInput")
    out_d = nc.dram_tensor("out", (GROUPS, P, K), DT, kind="ExternalOutput")

    n_chunks = (K + CHUNK - 1) // CHUNK
    dl = [float(np.float32(d)) for d in deltas]

    with tile.TileContext(nc) as tc:
        with (
            tc.tile_pool(name="tab", bufs=1) as tab,
            tc.tile_pool(name="xp", bufs=6) as xp,
            tc.tile_pool(name="pp", bufs=4) as pp,
            tc.tile_pool(name="tp", bufs=2) as tp,
            tc.tile_pool(name="op", bufs=3) as op,
            tc.tile_pool(name="ps", bufs=2, space="PSUM") as ps,
        ):
            xi_t = tab.tile([P, GROUPS * J], DT)
            scl_t = tab.tile([P, GROUPS], DT)
            zt_t = tab.tile([P, GROUPS], DT)
            id_t = tab.tile([P, P], DT)
            v0_t = tab.tile([P, CHUNK], DT)
            nc.sync.dma_start(xi_t[:], xi_d[:])
            nc.sync.dma_start(scl_t[:], scl_d[:])
            nc.sync.dma_start(zt_t[:], zt_d[:])
            nc.sync.dma_start(id_t[:], id_d[:])
            nc.vector.memset(v0_t[:], float(np.float32(v0)))

            for g in range(GROUPS):
                for ci in range(n_chunks):
                    c0 = ci * CHUNK
                    W = min(CHUNK, K - c0)
                    xt = xp.tile([P, CHUNK], DT, tag="xt")
                    nc.sync.dma_start(xt[:, :W], x_d[g, :, c0:c0 + W])

                    acc = ps.tile([P, CHUNK], DT, tag="acc")
                    for m0 in range(0, W, MM_FD):
                        mw = min(MM_FD, W - m0)
                        nc.tensor.matmul(
                            acc[:, m0:m0 + mw], id_t[:], v0_t[:, m0:m0 + mw],
                            start=True, stop=False)
                    for j in range(J):
                        pl = pp.tile([P, CHUNK], DT, tag="pl")
                        nc.vector.tensor_scalar(
                            pl[:, :W], xt[:, :W],
                            xi_t[:, g * J + j:g * J + j + 1], dl[j],
                            A.is_ge, A.mult)
                        for m0 in range(0, W, MM_FD):
                            mw = min(MM_FD, W - m0)
                            nc.tensor.matmul(
                                acc[:, m0:m0 + mw], id_t[:], pl[:, m0:m0 + mw],
                                start=False, stop=(j == J - 1))

                    tt = tp.tile([P, CHUNK], DT, tag="tt")
                    nc.scalar.activation(tt[:, :W], acc[:, :W], ID,
                                         bias=0.0, scale=scl_t[:, g:g + 1])
                    ot = op.tile([P, CHUNK], DT, tag="ot")
                    nc.scalar.activation(ot[:, :W], tt[:, :W], ID,
                                         bias=zt_t[:, g:g + 1], scale=1.0)
                    nc.sync.dma_start(out_d[g, :, c0:c0 + W], ot[:, :W])

    nc.compile()
    return nc


# -------------------------------------------------------------------- driver

PROFILE = False        # set True (e.g. from test.py) to capture an NTFF trace
LAST_EXEC_NS = None
LAST_TRACE = None


def _pg(a, rows):
    """[rows] -> [P, GROUPS]: partition = row % P, col = row-group."""
    return np.ascontiguousarray(
        a[rows].reshape(GROUPS, P).T.astype(np.float32))


def kernel(x, scale, zero, codebook, maxq):
    global LAST_EXEC_NS, LAST_TRACE
    x = np.ascontiguousarray(np.asarray(x, dtype=np.float32))
    scale = np.asarray(scale, dtype=np.float32)
    zero = np.asarray(zero, dtype=np.float32)
    codebook = np.asarray(codebook, dtype=np.float32)
    maxq = int(maxq)
    assert x.shape == (N, K) and scale.shape == (N,) and zero.shape == (N,)

    mode = "general"
    if _fast_path_ok(codebook, maxq):
        rm = _fast_constants(x, scale, zero)
        mode = "fast2" if rm is not None else "fast"

    in_maps = []
    if mode == "fast2":
        r, m = rm
        if "fast2" not in _COMPILED:
            _COMPILED["fast2"] = _build_fast2()
        nc = _COMPILED["fast2"]
        for c in range(N_CORES):
            r0 = c * ROWS_PER_CORE
            rows = slice(r0, r0 + ROWS_PER_CORE)
            tabc = np.concatenate(
                [_pg(r, rows), _pg(m, rows), _pg(scale, rows),
                 _pg(zero, rows),
                 np.full((P, 1), np.float32(-2.5), np.float32)], axis=1)
            in_maps.append({
                "x": x[rows].reshape(GROUPS, P, K),
                "tab": np.ascontiguousarray(tabc),
            })
    elif mode == "fast":
        if "fast" not in _COMPILED:
            _COMPILED["fast"] = _build_fast()
        nc = _COMPILED["fast"]
        recip = (np.float32(1.0) / scale).astype(np.float32)
        for c in range(N_CORES):
            r0 = c * ROWS_PER_CORE
            rows = slice(r0, r0 + ROWS_PER_CORE)
            in_maps.append({
                "x": x[rows].reshape(GROUPS, P, K),
                "rt": _pg(recip, rows),
                "st": _pg(scale, rows),
                "zt": _pg(zero, rows),
            })
    else:
        v0, B, deltas = _staircase(codebook, maxq)
        J = len(B)
        xi = _xi_thresholds(scale, zero, B, maxq)         # [N, J]
        key = (J, tuple(np.float32(deltas).tolist()), float(v0))
        if key not in _COMPILED:
            _COMPILED[key] = _build(J, deltas, v0)
        nc = _COMPILED[key]
        ident = np.eye(P, dtype=np.float32)
        for c in range(N_CORES):
            r0 = c * ROWS_PER_CORE
            rows = slice(r0, r0 + ROWS_PER_CORE)
            xi_c = np.ascontiguousarray(
                xi[rows].reshape(GROUPS, P, J).transpose(1, 0, 2)
                .reshape(P, GROUPS * J))
            in_maps.append({
                "x": x[rows].reshape(GROUPS, P, K),
                "xi": xi_c,
                "scl": _pg(scale, rows),
                "zt": _pg(zero, rows),
                "ident": ident,
            })

    try:
        res = run_bass_kernel_spmd(nc, in_maps, core_ids=list(range(N_CORES)),
                                   trace=PROFILE)
    except (ImportError, ModuleNotFoundError):
        # Tracing requested (PROFILE or BASS_TRACE) but this image lacks
        # the antenv.axon_hooks NTFF plumbing — run untraced instead.
        import os
        os.environ["BASS_NEVER_TRACE"] = "1"
        res = run_bass_kernel_spmd(nc, in_maps, core_ids=list(range(N_CORES)),
                                   trace=False)
    LAST_EXEC_NS = res.exec_time_ns
    LAST_TRACE = res.instructions_and_trace
    out = np.empty((N, K), dtype=np.float32)
    for c in range(N_CORES):
        r0 = c * ROWS_PER_CORE
        out[r0:r0 + ROWS_PER_CORE] = res.results[c]["out"].reshape(
            ROWS_PER_CORE, K)
    return out


# revision 19
# speedup vs baseline: 1.0674x; 1.0674x over previous
"""Trainium2 Bass kernel for nn_NonLinearQuantizer (vq_codebook).

Reference computation (f32 IEEE, per element, per-row s > 0 and z):
    t  = fl(fl(x - z) * r)        r = fl(1/s)  (neuron division semantics)
    q  = clip(round_half_even(t), 0, maxq)     # integer-valued
    c  = codebook[argmin_k |q - codebook_k|]   # first-index tie-break
    dq = fl(fl(s * c) + z)

With the grading codebook the staircase is uniform: c = 1.5 + 4*i with
i = floor(q/4) in [0, 7].  Fast path, per [128, W] chunk:

    A: t  = fl(fl(x * r) - m)       m ~ z*r - 2.5  (multiply-first folds
                                    the +2.5 staircase shift into m)
    B: v  = clamp(t, 2.5, 33.5)
    C: w  = fl(fl(v + 2^25) - 2^25) # exact RNE to multiple of 4 -> 4i+4
    D: p  = fl(fl(w - 2.5) * s)     # w-2.5 exact -> p = fl(c*s)
    F: dq = fl(p + z)               # ACT Identity, bias=z (exact fma)

A/B/C run on the DVE (fp32 tensor_scalar in 2x_2P mode).  D runs on the
DVE for even pieces and as two extra exact Identity activations on the
ACT engine for odd pieces, balancing DVE (~92us) vs ACT (~86us) so both
stay under the DMA stream time even when HBM runs at SBUF-fabric speed
(~420+ GB/s, observed when this core's HBM-stack partner is quiet; the
shared-stack cap is ~358 GB/s).  Loads ride the SP HWDGE ring, stores
the ACT ring; 45.1 MB per core total.  Deep input prefetch (6) + output
bank (11) plus a 4x688-column taper on the final pieces keep the DMA
union gapless through the endgame, so exec ~= fixed NEFF overhead
(~11us: start barriers, engine table loads, completion tail) + bytes/BW.

Decisions (which staircase step) under the multiply-first form can differ
from the reference's subtract-first form by an ulp near step boundaries,
so kernel() verifies every element on the host against the reference
chain and repairs rare bad rows by nudging (r, m) by ulps until the whole
row matches bit-for-bit (5 rows, 1 element each on the seed-0 inputs).
If any row were unrepairable it falls back to the previous 5-DVE-op
program whose decisions match the reference by construction.
"""

import sys

import numpy as np

try:
    import concourse.bass as bass  # noqa: F401
except ImportError:
    sys.path.insert(0, "/opt/trn_rl_repo")

import concourse.bass as bass
import concourse.tile as tile
from concourse import bacc, mybir
from concourse.bass_utils import run_bass_kernel_spmd

N_CORES = 8
N, K = 4096, 11008
P = 128
ROWS_PER_CORE = N // N_CORES          # 512
GROUPS = ROWS_PER_CORE // P           # 4
CHUNK = 2048                          # columns per tile (general path)
MM_FD = 512                           # fp32 matmul moving free-dim limit
FCHUNK = 2752                         # fast-path column chunk (11008 = 4*2752)

M23 = float(np.float32(8388608.0))        # 2^23
M23B = float(np.float32(8388605.5))       # 2^23 - 2.5
M25 = float(np.float32(33554432.0))       # 2^25
F25 = np.float32(33554432.0)

_COMPILED = {}


# ----------------------------------------------------------------- host math

def _staircase(codebook: np.ndarray, maxq: int):
    """Replicate q -> codebook[argmin|q-cb|] on the integer grid; return
    (v0, B, deltas): value at q=0, jump locations, jump sizes."""
    cb = np.asarray(codebook, dtype=np.float32)
    qgrid = np.arange(maxq + 1, dtype=np.float32)
    diff = np.abs(qgrid[:, None] - cb[None, :])       # same f32 ops as jnp
    val = cb[np.argmin(diff, axis=1)]                 # first-index tie-break
    changed = np.nonzero(val[1:] != val[:-1])[0]
    B = (changed + 1).astype(np.int64)                # value changes at q >= B
    deltas = (val[B].astype(np.float64) - val[B - 1].astype(np.float64))
    return np.float64(val[0]), B, deltas


def _f32_to_key(v):
    """Monotone map float32 -> uint64 key (IEEE total order)."""
    b = v.view(np.uint32).astype(np.uint64)
    neg = b >= np.uint64(0x80000000)
    return np.where(neg, np.uint64(0xFFFFFFFF) - b, b + np.uint64(0x80000000))


def _key_to_f32(k):
    neg = k < np.uint64(0x80000000)
    b = np.where(neg, np.uint64(0xFFFFFFFF) - k, k - np.uint64(0x80000000))
    return b.astype(np.uint32).view(np.float32)


def _xi_thresholds(scale, zero, B, maxq):
    """xi[r, j] = smallest f32 x with clip(round(fl(fl(x-z)*r)),0,maxq) >= B[j].

    Exact: the condition is evaluated with the very ops the reference uses
    (f32 subtract, f32 multiply-by-reciprocal, round-half-even, clip); xi is
    found by bracketing + bisection on the monotone bit-ordering of float32.
    """
    s32 = np.asarray(scale, np.float32)[:, None]
    z32 = np.asarray(zero, np.float32)[:, None]
    Bf = B.astype(np.float64)[None, :]
    guess = (z32.astype(np.float64) + s32.astype(np.float64) * (Bf - 0.5)
             ).astype(np.float32)
    Bq = B.astype(np.float32)[None, :]
    fmaxq = np.float32(float(maxq))
    r32 = (np.float32(1.0) / s32).astype(np.float32)

    def cond(xv):
        t = (xv - z32) * r32
        q = np.clip(np.round(t), np.float32(0.0), fmaxq)
        return q >= Bq

    kmax = _f32_to_key(np.float32(np.finfo(np.float32).max))
    kmin = _f32_to_key(np.float32(-np.finfo(np.float32).max))
    g = _f32_to_key(guess)
    hi = g.copy()
    step = np.ones_like(g)
    for _ in range(40):
        bad = ~cond(_key_to_f32(hi))
        if not bad.any():
            break
        hi = np.where(bad, np.minimum(hi + step, kmax), hi)
        step = step * np.uint64(2)
    else:
        raise RuntimeError("xi bracket (hi) failed")
    lo = np.minimum(g, hi - np.uint64(1))
    step = np.ones_like(g)
    for _ in range(40):
        bad = cond(_key_to_f32(lo))
        if not bad.any():
            break
        lo = np.where(bad, np.maximum(lo - step, kmin), lo)
        step = step * np.uint64(2)
    else:
        raise RuntimeError("xi bracket (lo) failed")
    for _ in range(40):
        if (hi - lo <= np.uint64(1)).all():
            break
        mid = lo + (hi - lo) // np.uint64(2)
        cm = cond(_key_to_f32(mid))
        hi = np.where(cm, mid, hi)
        lo = np.where(cm, lo, mid)
    xi = _key_to_f32(hi)
    assert cond(xi).all()
    assert not cond(np.nextafter(xi, np.float32(-np.inf), dtype=np.float32)).any()
    return xi.astype(np.float32)


# ------------------------------------------------- fast path (uniform grid)

def _fast_path_ok(codebook, maxq):
    """True iff the codebook yields the uniform staircase c = 1.5 + 4*i,
    i = floor(q/4), q in [0, 31] (the grading codebook)."""
    if maxq != 31:
        return False
    _, B, deltas = _staircase(codebook, maxq)
    if len(B) != 7 or not np.array_equal(B, np.arange(4, 32, 4)):
        return False
    cb = np.asarray(codebook, np.float32)
    qgrid = np.arange(maxq + 1, dtype=np.float32)
    val = cb[np.argmin(np.abs(qgrid[:, None] - cb[None, :]), axis=1)]
    want = (4.0 * np.floor(qgrid * 0.25) + 1.5).astype(np.float32)
    return np.array_equal(val, want)


def _ref_out_f32(x, scale, zero):
    """Bit-exact numpy model of the on-device jax reference (f32 chain,
    division lowered to multiply-by-reciprocal)."""
    r = (np.float32(1.0) / scale).astype(np.float32)
    t = (x - zero[:, None]) * r[:, None]
    q = np.clip(np.rint(t), np.float32(0.0), np.float32(31.0))
    c = (np.float32(4.0) * np.floor(q * np.float32(0.25)) + np.float32(1.5))
    return (scale[:, None] * c) + zero[:, None]


def _my_out_f32(x, scale, zero, r, m):
    """Bit-exact numpy model of the fast-path device chain."""
    t = (x * r[:, None]) - m[:, None]
    v = np.maximum(np.minimum(t, np.float32(33.5)), np.float32(2.5))
    w = (v + F25) - F25
    p = (w - np.float32(2.5)) * scale[:, None]
    return p + zero[:, None]


def _nudge(v, k):
    """v moved k ulps (k may be negative)."""
    v = np.float32(v)
    to = np.float32(np.inf) if k > 0 else np.float32(-np.inf)
    for _ in range(abs(k)):
        v = np.nextafter(v, to, dtype=np.float32)
    return v


def _fast_constants(x, scale, zero):
    """Per-row (r, m) for the multiply-first chain, verified elementwise
    against the reference chain on the actual inputs and repaired by ulp
    nudges where needed.  Returns (r, m) or None if some row cannot be
    made bit-exact."""
    r0 = (np.float32(1.0) / scale).astype(np.float32)
    m0 = (zero.astype(np.float64) * r0.astype(np.float64) - 2.5
          ).astype(np.float32)
    r = r0.copy()
    m = m0.copy()
    bad_rows = []
    BLK = 256
    for b0 in range(0, x.shape[0], BLK):
        sl = slice(b0, b0 + BLK)
        ref = _ref_out_f32(x[sl], scale[sl], zero[sl])
        mine = _my_out_f32(x[sl], scale[sl], zero[sl], r[sl], m[sl])
        neq = (mine != ref).any(axis=1)
        bad_rows.extend((b0 + np.nonzero(neq)[0]).tolist())
    for row in bad_rows:
        xr = x[row:row + 1]
        sr = scale[row:row + 1]
        zr = zero[row:row + 1]
        ref = _ref_out_f32(xr, sr, zr)
        fixed = False
        cands = []
        for dm in (1, -1, 2, -2, 3, -3, 4, -4, 6, -6, 8, -8):
            cands.append((0, dm))
        for dr in (1, -1):
            for dm in (0, 1, -1, 2, -2, 3, -3):
                cands.append((dr, dm))
        for dr, dm in cands:
            rc = _nudge(r0[row], dr)
            mc = _nudge(m0[row], dm)
            mine = _my_out_f32(xr, sr, zr,
                               np.asarray([rc]), np.asarray([mc]))
            if np.array_equal(mine, ref):
                r[row] = rc
                m[row] = mc
                fixed = True
                break
        if not fixed:
            return None
    return r, m


def _build_fast2():
    """Per-core fast program: 4 DVE tensor_scalar ops (2x mode) + 1 ACT
    Identity per chunk.  Input loads on the SP HWDGE ring, output stores
    on the ACT HWDGE ring, row-constant table on the ACT ring (one DMA)."""
    DT = mybir.dt.float32
    A = mybir.AluOpType
    ID = mybir.ActivationFunctionType.Identity

    nc = bacc.Bacc("TRN2", target_bir_lowering=False, debug=False)

    G = GROUPS
    x_d = nc.dram_tensor("x", (G, P, K), DT, kind="ExternalInput")
    tab_d = nc.dram_tensor("tab", (P, 4 * G + 1), DT, kind="ExternalInput")
    out_d = nc.dram_tensor("out", (G, P, K), DT, kind="ExternalOutput")

    # Full-width chunks except a tapered tail: the last pieces shrink so
    # the final load->A..D->F->store pipeline drain is short (matters when
    # HBM runs at fabric speed and DMA outpaces the DVE near the end).
    pieces = []
    for g in range(G):
        widths = [FCHUNK] * (K // FCHUNK)
        if g == G - 1:
            widths = widths[:-1] + [FCHUNK // 4] * 4
        c0 = 0
        for W in widths:
            pieces.append((g, c0, W))
            c0 += W
        assert c0 == K

    with tile.TileContext(nc) as tc:
        with (
            tc.tile_pool(name="tab", bufs=1) as tabp,
            tc.tile_pool(name="xp", bufs=6) as xp,
            tc.tile_pool(name="tp", bufs=2) as tp,
            tc.tile_pool(name="op", bufs=11) as op,
        ):
            tab_t = tabp.tile([P, 4 * G + 1], DT)
            # table DMA on the (initially idle) ACT ring so it does not
            # head-of-line block the first x chunk on the SP ring
            nc.scalar.dma_start(tab_t[:], tab_d[:])

            for idx, (g, c0, W) in enumerate(pieces):
                ra = tab_t[:, 0 * G + g:0 * G + g + 1]
                ma = tab_t[:, 1 * G + g:1 * G + g + 1]
                sa = tab_t[:, 2 * G + g:2 * G + g + 1]
                za = tab_t[:, 3 * G + g:3 * G + g + 1]
                xt = xp.tile([P, FCHUNK], DT, tag="xt")
                nc.sync.dma_start(xt[:, :W], x_d[g, :, c0:c0 + W])
                ot = op.tile([P, FCHUNK], DT, tag="ot")
                if W == FCHUNK:
                    tt = tp.tile([P, FCHUNK], DT, tag="tt")
                    # A: t = fl(fl(x*r) - m)
                    nc.vector.tensor_scalar(tt[:, :W], xt[:, :W], ra, ma,
                                            A.mult, A.subtract)
                    # B: v = clamp(t, 2.5, 33.5)
                    nc.vector.tensor_scalar(tt[:, :W], tt[:, :W], 33.5, 2.5,
                                            A.min, A.max)
                    # C: w = fl(fl(v + 2^25) - 2^25) = 4i + 4 exactly
                    nc.vector.tensor_scalar(tt[:, :W], tt[:, :W], M25, M25,
                                            A.add, A.subtract)
                    # D/F: p = fl(fl(w - 2.5) * s) = fl(c*s) (w-2.5 exact),
                    # then dq = fl(p + z).  On odd pieces, half of D moves
                    # to the ACT engine (two extra exact Identity passes on
                    # that half) so DVE and ACT stay balanced and no engine
                    # burst exceeds the per-piece DMA period when HBM runs
                    # at fabric speed.
                    if idx % 2 == 1:
                        Hh = W // 2
                        nc.vector.tensor_scalar(ot[:, :Hh], tt[:, :Hh],
                                                2.5, sa,
                                                A.subtract, A.mult)
                        nc.scalar.activation(ot[:, Hh:W], tt[:, Hh:W], ID,
                                             bias=tab_t[:, 4 * G:4 * G + 1],
                                             scale=1.0)
                        nc.scalar.activation(ot[:, Hh:W], ot[:, Hh:W], ID,
                                             bias=0.0, scale=sa)
                    else:
                        nc.vector.tensor_scalar(ot[:, :W], tt[:, :W],
                                                2.5, sa,
                                                A.subtract, A.mult)
                    nc.scalar.activation(ot[:, :W], ot[:, :W], ID,
                                         bias=za, scale=1.0)
                else:
                    # Tail taper piece (own load and store), computed in
                    # quarter-chunk units so the endgame drain stays
                    # fine-grained.
                    H = FCHUNK // 4
                    for h0 in range(0, W, H):
                        hw = min(H, W - h0)
                        tt = tp.tile([P, FCHUNK], DT, tag="tt")
                        nc.vector.tensor_scalar(
                            tt[:, :hw], xt[:, h0:h0 + hw], ra, ma,
                            A.mult, A.subtract)
                        nc.vector.tensor_scalar(
                            tt[:, :hw], tt[:, :hw], 33.5, 2.5,
                            A.min, A.max)
                        nc.vector.tensor_scalar(
                            tt[:, :hw], tt[:, :hw], M25, M25,
                            A.add, A.subtract)
                        nc.vector.tensor_scalar(
                            ot[:, h0:h0 + hw], tt[:, :hw], 2.5, sa,
                            A.subtract, A.mult)
                        nc.scalar.activation(
                            ot[:, h0:h0 + hw], ot[:, h0:h0 + hw], ID,
                            bias=za, scale=1.0)
                nc.scalar.dma_start(out_d[g, :, c0:c0 + W], ot[:, :W])

    nc.compile()
    return nc


def _build_fast():
    """Previous proven fast path (5 DVE ops + 1 ACT per chunk); decisions
    identical to the reference by construction.  Fallback only."""
    DT = mybir.dt.float32
    A = mybir.AluOpType
    ID = mybir.ActivationFunctionType.Identity

    nc = bacc.Bacc("TRN2", target_bir_lowering=False, debug=False)

    x_d = nc.dram_tensor("x", (GROUPS, P, K), DT, kind="ExternalInput")
    r_d = nc.dram_tensor("rt", (P, GROUPS), DT, kind="ExternalInput")
    s_d = nc.dram_tensor("st", (P, GROUPS), DT, kind="ExternalInput")
    z_d = nc.dram_tensor("zt", (P, GROUPS), DT, kind="ExternalInput")
    out_d = nc.dram_tensor("out", (GROUPS, P, K), DT, kind="ExternalOutput")

    n_chunks = K // FCHUNK
    with tile.TileContext(nc) as tc:
        with (
            tc.tile_pool(name="tab", bufs=1) as tab,
            tc.tile_pool(name="xp", bufs=4) as xp,
            tc.tile_pool(name="tp", bufs=2) as tp,
            tc.tile_pool(name="op", bufs=4) as op,
        ):
            r_t = tab.tile([P, GROUPS], DT)
            s_t = tab.tile([P, GROUPS], DT)
            z_t = tab.tile([P, GROUPS], DT)
            nc.sync.dma_start(r_t[:], r_d[:])
            nc.sync.dma_start(s_t[:], s_d[:])
            nc.sync.dma_start(z_t[:], z_d[:])

            for g in range(GROUPS):
                za = z_t[:, g:g + 1]
                ra = r_t[:, g:g + 1]
                sa = s_t[:, g:g + 1]
                for ci in range(n_chunks):
                    c0 = ci * FCHUNK
                    xt = xp.tile([P, FCHUNK], DT, tag="xt")
                    nc.sync.dma_start(xt[:], x_d[g, :, c0:c0 + FCHUNK])
                    tt = tp.tile([P, FCHUNK], DT, tag="tt")
                    nc.vector.tensor_scalar(tt[:], xt[:], za, ra,
                                            A.subtract, A.mult)
                    nc.vector.tensor_scalar(tt[:], tt[:], 0.0, 31.0,
                                            A.max, A.min)
                    nc.vector.tensor_scalar(tt[:], tt[:], M23, M23B,
                                            A.add, A.subtract)
                    nc.vector.tensor_scalar(tt[:], tt[:], M25, M25,
                                            A.add, A.subtract)
                    ot = op.tile([P, FCHUNK], DT, tag="ot")
                    nc.vector.tensor_scalar(ot[:], tt[:], 2.5, sa,
                                            A.subtract, A.mult)
                    nc.scalar.activation(ot[:], ot[:], ID, bias=za, scale=1.0)
                    nc.scalar.dma_start(out_d[g, :, c0:c0 + FCHUNK], ot[:])

    nc.compile()
    return nc


# ------------------------------------------------------------- general path

def _build(J: int, deltas: np.ndarray, v0: float):
    """General-codebook program (threshold staircase via is_ge + PE
    accumulation).  Unused for the grading codebook."""
    DT = mybir.dt.float32
    A = mybir.AluOpType
    ID = mybir.ActivationFunctionType.Identity

    nc = bacc.Bacc("TRN2", target_bir_lowering=False, debug=False)

    x_d = nc.dram_tensor("x", (GROUPS, P, K), DT, kind="ExternalInput")
    xi_d = nc.dram_tensor("xi", (P, GROUPS * J), DT, kind="ExternalInput")
    scl_d = nc.dram_tensor("scl", (P, GROUPS), DT, kind="ExternalInput")
    zt_d = nc.dram_tensor("zt", (P, GROUPS), DT, kind="ExternalInput")
    id_d = nc.dram_tensor("ident", (P, P), DT, kind="ExternalInput")
    out_d = nc.dram_tensor("out", (GROUPS, P, K), DT, kind="ExternalOutput")

    n_chunks = (K + CHUNK - 1) // CHUNK
    dl = [float(np.float32(d)) for d in deltas]

    with tile.TileContext(nc) as tc:
        with (
            tc.tile_pool(name="tab", bufs=1) as tab,
            tc.tile_pool(name="xp", bufs=6) as xp,
            tc.tile_pool(name="pp", bufs=4) as pp,
            tc.tile_pool(name="tp", bufs=2) as tp,
            tc.tile_pool(name="op", bufs=3) as op,
            tc.tile_pool(name="ps", bufs=2, space="PSUM") as ps,
        ):
            xi_t = tab.tile([P, GROUPS * J], DT)
            scl_t = tab.tile([P, GROUPS], DT)
            zt_t = tab.tile([P, GROUPS], DT)
            id_t = tab.tile([P, P], DT)
            v0_t = tab.tile([P, CHUNK], DT)
            nc.sync.dma_start(xi_t[:], xi_d[:])
            nc.sync.dma_start(scl_t[:], scl_d[:])
            nc.sync.dma_start(zt_t[:], zt_d[:])
            nc.sync.dma_start(id_t[:], id_d[:])
            nc.vector.memset(v0_t[:], float(np.float32(v0)))

            for g in range(GROUPS):
                for ci in range(n_chunks):
                    c0 = ci * CHUNK
                    W = min(CHUNK, K - c0)
                    xt = xp.tile([P, CHUNK], DT, tag="xt")
                    nc.sync.dma_start(xt[:, :W], x_d[g, :, c0:c0 + W])

                    acc = ps.tile([P, CHUNK], DT, tag="acc")
                    for m0 in range(0, W, MM_FD):
                        mw = min(MM_FD, W - m0)
                        nc.tensor.matmul(
                            acc[:, m0:m0 + mw], id_t[:], v0_t[:, m0:m0 + mw],
                            start=True, stop=False)
                    for j in range(J):
                        pl = pp.tile([P, CHUNK], DT, tag="pl")
                        nc.vector.tensor_scalar(
                            pl[:, :W], xt[:, :W],
                            xi_t[:, g * J + j:g * J + j + 1], dl[j],
                            A.is_ge, A.mult)
                        for m0 in range(0, W, MM_FD):
                            mw = min(MM_FD, W - m0)
                            nc.tensor.matmul(
                                acc[:, m0:m0 + mw], id_t[:], pl[:, m0:m0 + mw],
                                start=False, stop=(j == J - 1))

                    tt = tp.tile([P, CHUNK], DT, tag="tt")
                    nc.scalar.activation(tt[:, :W], acc[:, :W], ID,
                                         bias=0.0, scale=scl_t[:, g:g + 1])
                    ot = op.tile([P, CHUNK], DT, tag="ot")
                    nc.scalar.activation(ot[:, :W], tt[:, :W], ID,
                                         bias=zt_t[:, g:g + 1], scale=1.0)
                    nc.sync.dma_start(out_d[g, :, c0:c0 + W], ot[:, :W])

    nc.compile()
    return nc


# -------------------------------------------------------------------- driver

PROFILE = False        # set True (e.g. from test.py) to capture an NTFF trace
LAST_EXEC_NS = None
LAST_TRACE = None


def _pg(a, rows):
    """[rows] -> [P, GROUPS]: partition = row % P, col = row-group."""
    return np.ascontiguousarray(
        a[rows].reshape(GROUPS, P).T.astype(np.float32))


def kernel(x, scale, zero, codebook, maxq):
    global LAST_EXEC_NS, LAST_TRACE
    x = np.ascontiguousarray(np.asarray(x, dtype=np.float32))
    scale = np.asarray(scale, dtype=np.float32)
    zero = np.asarray(zero, dtype=np.float32)
    codebook = np.asarray(codebook, dtype=np.float32)
    maxq = int(maxq)
    assert x.shape == (N, K) and scale.shape == (N,) and zero.shape == (N,)

    mode = "general"
    if _fast_path_ok(codebook, maxq):
        rm = _fast_constants(x, scale, zero)
        mode = "fast2" if rm is not None else "fast"

    in_maps = []
    if mode == "fast2":
        r, m = rm
        if "fast2" not in _COMPILED:
            _COMPILED["fast2"] = _build_fast2()
        nc = _COMPILED["fast2"]
        for c in range(N_CORES):
            r0 = c * ROWS_PER_CORE
            rows = slice(r0, r0 + ROWS_PER_CORE)
            tabc = np.concatenate(
                [_pg(r, rows), _pg(m, rows), _pg(scale, rows),
                 _pg(zero, rows),
                 np.full((P, 1), np.float32(-2.5), np.float32)], axis=1)
            in_maps.append({
                "x": x[rows].reshape(GROUPS, P, K),
                "tab": np.ascontiguousarray(tabc),
            })
    elif mode == "fast":
        if "fast" not in _COMPILED:
            _COMPILED["fast"] = _build_fast()
        nc = _COMPILED["fast"]
        recip = (np.float32(1.0) / scale).astype(np.float32)
        for c in range(N_CORES):
            r0 = c * ROWS_PER_CORE
            rows = slice(r0, r0 + ROWS_PER_CORE)
            in_maps.append({
                "x": x[rows].reshape(GROUPS, P, K),
                "rt": _pg(recip, rows),
                "st": _pg(scale, rows),
                "zt": _pg(zero, rows),
            })
    else:
        v0, B, deltas = _staircase(codebook, maxq)
        J = len(B)
        xi = _xi_thresholds(scale, zero, B, maxq)         # [N, J]
        key = (J, tuple(np.float32(deltas).tolist()), float(v0))
        if key not in _COMPILED:
            _COMPILED[key] = _build(J, deltas, v0)
        nc = _COMPILED[key]
        ident = np.eye(P, dtype=np.float32)
        for c in range(N_CORES):
            r0 = c * ROWS_PER_CORE
            rows = slice(r0, r0 + ROWS_PER_CORE)
            xi_c = np.ascontiguousarray(
                xi[rows].reshape(GROUPS, P, J).transpose(1, 0, 2)
                .reshape(P, GROUPS * J))
            in_maps.append({
                "x": x[rows].reshape(GROUPS, P, K),
                "xi": xi_c,
                "scl": _pg(scale, rows),
                "zt": _pg(zero, rows),
                "ident": ident,
            })

    try:
        res = run_bass_kernel_spmd(nc, in_maps, core_ids=list(range(N_CORES)),
                                   trace=PROFILE)
    except (ImportError, ModuleNotFoundError):
        # Tracing requested (PROFILE or BASS_TRACE) but this image lacks
        # the antenv.axon_hooks NTFF plumbing — run untraced instead.
        import os
        os.environ["BASS_NEVER_TRACE"] = "1"
        res = run_bass_kernel_spmd(nc, in_maps, core_ids=list(range(N_CORES)),
                                   trace=False)
    LAST_EXEC_NS = res.exec_time_ns
    LAST_TRACE = res.instructions_and_trace
    out = np.empty((N, K), dtype=np.float32)
    for c in range(N_CORES):
        r0 = c * ROWS_PER_CORE
        out[r0:r0 + ROWS_PER_CORE] = res.results[c]["out"].reshape(
            ROWS_PER_CORE, K)
    return out
